# revision 1
# baseline (speedup 1.0000x reference)
"""Trainium2 Bass kernel: 3x3 valid cross-correlation (dense CNN layer).

  x:       (128, 224, 224) f32   (C_in, H, W)
  kernels: (256, 128, 3, 3) f32  (C_out, C_in, KH, KW)
  out:     (256, 222, 222) f32   (C_out, H_out, W_out)

Sharding: output rows spatially across the 8 NeuronCores (28 rows per core;
8*28 = 224 >= 222, tail rows computed from zero padding and dropped on
gather). Every core holds the full filter bank. C_in = 128 is exactly the PE
contraction dim; output channels form two 128-partition groups. For each
(row-pair, channel-group) a PSUM tile (128, 2, 222) accumulates one matmul
per filter tap, the moving operand being a shifted window of the SBUF-
resident input slab.

Precision modes (CONV_MM_MODE):
  f32r3 (default): fp32r hi/lo split. fp32r is fp32 RNE-rounded to 11
      explicit mantissa bits but streams at full PE rate; the matmul is
      exact (fp32 PSUM accumulate) on fp32r-representable values. With
      x = x_hi + x_lo and w = w_hi + w_lo (both halves exactly
      representable), 3 passes (hi*hi + hi*lo + lo*hi) give full-fp32
      accuracy at 3 cycles/row vs plain fp32's 4.
  f32r: single-pass fp32r (~1.5e-4 rel err, 1 cycle/row).
  f32:  plain fp32 matmul (4 cycles/row).
  bf16: single-pass bf16 (~1e-2 rel err, 1 cycle/row).
"""

import os
from contextlib import ExitStack

import numpy as np

C_IN, H, W = 128, 224, 224
C_OUT, KH, KW = 256, 3, 3
H_OUT = H - KH + 1  # 222
W_OUT = W - KW + 1  # 222
N_CORES = 8
ROWS_PER_CORE = 28
IN_ROWS = ROWS_PER_CORE + KH - 1  # 30
PAIRS = ROWS_PER_CORE // 2  # 14
N_GROUPS = C_OUT // 128  # 2
H_PAD = N_CORES * ROWS_PER_CORE + KH - 1  # 226
N_TAPS = KH * KW  # 9

MM_MODE = os.environ.get("CONV_MM_MODE", "f32r3")

_compiled = {}


def _round_f32r(a):
    """fp32 -> nearest fp32r (low 12 mantissa bits dropped, RNE) — the exact
    rounding trn2 applies when data is ingested as dt.float32r."""
    b = a.view(np.uint32).astype(np.uint64)
    q = np.uint64(1 << 12)
    r = (b + ((q >> np.uint64(1)) - np.uint64(1) + ((b >> np.uint64(12)) & np.uint64(1)))) & ~(q - np.uint64(1))
    return r.astype(np.uint32).view(np.float32)


DEFAULT_CFG = dict(
    xp_bufs=1,
    wp_bufs=1,
    op_bufs=8,
    pp_bufs=8,
    # term-major matmul order + interleaved hi/lo input chunks + per-group w
    # chunks minimize the pipeline-fill stall at kernel start (the first 9
    # matmuls only need w_hi[g0] and the first x chunks). Sustained slope is
    # PE-bound and config-insensitive; these help the single-shot case.
    term_major=True,
    x_chunk=6,
    w_group_chunks=True,
    x_h_outer=False,
    in_dma_gpsimd=False,  # issue input DMAs from gpsimd (separate queues from output)
    pair_block=0,  # >0: tap-major over a block of row-pairs sharing each weight
    # ablation flags (repeat-loop timing experiments)
    load_in_loop=True,  # False: hoist x/w DMA out of the repeat loop
    do_copy=True,  # False: skip psum->sbuf copy except an anchor on the last tile
    do_store=True,  # False: skip output DMA
)


def _build(mm_mode, repeat=1, **cfg_over):
    import concourse.mybir as mybir
    import concourse.tile as tile
    from concourse import bacc

    cfg = {**DEFAULT_CFG, **cfg_over}
    dt = mybir.dt
    split = mm_mode == "f32r3"
    mm_dt = {
        "f32r3": dt.float32r,
        "f32r": dt.float32r,
        "f32": dt.float32,
        "bf16": dt.bfloat16,
    }[mm_mode]
    n_half = 2 if split else 1  # hi/lo copies of x and w

    nc = bacc.Bacc("TRN2", target_bir_lowering=False)
    x_d = nc.dram_tensor(
        "x", [n_half, C_IN, IN_ROWS, W], mm_dt, kind="ExternalInput"
    ).ap()
    w_d = nc.dram_tensor(
        "w", [n_half, C_IN, N_GROUPS * N_TAPS, 128], mm_dt, kind="ExternalInput"
    ).ap()
    o_d = nc.dram_tensor(
        "out", [N_GROUPS, 128, ROWS_PER_CORE, W_OUT], dt.float32, kind="ExternalOutput"
    ).ap()

    def load(nc, tc, xp, wp):
        in_eng = nc.gpsimd if cfg["in_dma_gpsimd"] else nc.sync
        w_sb = wp.tile([C_IN, n_half * N_GROUPS * N_TAPS, 128], mm_dt, name="w_sb")
        if cfg["w_group_chunks"]:
            for h in range(n_half):
                for g in range(N_GROUPS):
                    in_eng.dma_start(
                        w_sb[
                            :,
                            h * N_GROUPS * N_TAPS + g * N_TAPS : h * N_GROUPS * N_TAPS
                            + (g + 1) * N_TAPS,
                            :,
                        ],
                        w_d[h, :, g * N_TAPS : (g + 1) * N_TAPS, :],
                    )
        else:
            for h in range(n_half):
                in_eng.dma_start(
                    w_sb[:, h * N_GROUPS * N_TAPS : (h + 1) * N_GROUPS * N_TAPS, :],
                    w_d[h],
                )
        x_sb = xp.tile([C_IN, n_half * IN_ROWS, W], mm_dt, name="x_sb")
        x_chunk = cfg["x_chunk"]
        if cfg["x_h_outer"]:
            for h in range(n_half):
                for r0 in range(0, IN_ROWS, x_chunk):
                    r1 = min(r0 + x_chunk, IN_ROWS)
                    in_eng.dma_start(
                        x_sb[:, h * IN_ROWS + r0 : h * IN_ROWS + r1, :],
                        x_d[h, :, r0:r1, :],
                    )
        else:
            for r0 in range(0, IN_ROWS, x_chunk):
                r1 = min(r0 + x_chunk, IN_ROWS)
                for h in range(n_half):
                    in_eng.dma_start(
                        x_sb[:, h * IN_ROWS + r0 : h * IN_ROWS + r1, :],
                        x_d[h, :, r0:r1, :],
                    )
        return w_sb, x_sb

    def compute(nc, tc, op, pp, w_sb, x_sb):
        # matmul passes per tap: (w_half, x_half)
        terms = [(0, 0), (0, 1), (1, 0)] if split else [(0, 0)]
        n_mm = len(terms) * N_TAPS
        taps = [(kh, kw) for kh in range(KH) for kw in range(KW)]
        if cfg["term_major"]:
            mm_order = [(wh, xh, kh, kw) for (wh, xh) in terms for (kh, kw) in taps]
        else:
            mm_order = [(wh, xh, kh, kw) for (kh, kw) in taps for (wh, xh) in terms]

        def emit_mm(ps, p, g, wh, xh, kh, kw, start, stop):
            nc.tensor.matmul(
                ps[:],
                w_sb[:, wh * N_GROUPS * N_TAPS + (g * KH + kh) * KW + kw, :],
                x_sb[
                    :,
                    xh * IN_ROWS + 2 * p + kh : xh * IN_ROWS + 2 * p + kh + 2,
                    kw : kw + W_OUT,
                ],
                start=start,
                stop=stop,
            )

        def emit_out(ps, p, g, last):
            if cfg["do_copy"] or last:
                ot = op.tile([128, 2, W_OUT], dt.float32, name="ot")
                nc.vector.tensor_copy(ot[:], ps[:])
                if cfg["do_store"] or last:
                    nc.sync.dma_start(o_d[g, :, 2 * p : 2 * p + 2, :], ot[:])

        B = cfg["pair_block"]
        if B:
            # consecutive matmuls share one stationary weight across B pairs
            for g in range(N_GROUPS):
                for b0 in range(0, PAIRS, B):
                    blk = list(range(b0, min(b0 + B, PAIRS)))
                    tiles = {
                        p: pp.tile([128, 2, W_OUT], dt.float32, name="ps") for p in blk
                    }
                    for i_mm, (wh, xh, kh, kw) in enumerate(mm_order):
                        for p in blk:
                            emit_mm(
                                tiles[p], p, g, wh, xh, kh, kw,
                                i_mm == 0, i_mm == n_mm - 1,
                            )
                    for p in blk:
                        emit_out(
                            tiles[p], p, g,
                            p == PAIRS - 1 and g == N_GROUPS - 1,
                        )
        else:
            for p in range(PAIRS):
                for g in range(N_GROUPS):
                    ps = pp.tile([128, 2, W_OUT], dt.float32, name="ps")
                    for i_mm, (wh, xh, kh, kw) in enumerate(mm_order):
                        emit_mm(ps, p, g, wh, xh, kh, kw, i_mm == 0, i_mm == n_mm - 1)
                    emit_out(ps, p, g, p == PAIRS - 1 and g == N_GROUPS - 1)

    with tile.TileContext(nc) as tc, ExitStack() as ctx:
        xp = ctx.enter_context(tc.tile_pool(name="xp", bufs=cfg["xp_bufs"]))
        wp = ctx.enter_context(tc.tile_pool(name="wp", bufs=cfg["wp_bufs"]))
        op = ctx.enter_context(tc.tile_pool(name="op", bufs=cfg["op_bufs"]))
        pp = ctx.enter_context(
            tc.tile_pool(name="pp", bufs=cfg["pp_bufs"], space="PSUM")
        )
        if repeat == 1:
            w_sb, x_sb = load(nc, tc, xp, wp)
            compute(nc, tc, op, pp, w_sb, x_sb)
        elif cfg["load_in_loop"]:
            with tc.For_i(0, repeat, 1):
                w_sb, x_sb = load(nc, tc, xp, wp)
                compute(nc, tc, op, pp, w_sb, x_sb)
        else:
            w_sb, x_sb = load(nc, tc, xp, wp)
            with tc.For_i(0, repeat, 1):
                compute(nc, tc, op, pp, w_sb, x_sb)

    nc.compile()
    return nc


def _get_nc(mode):
    if mode not in _compiled:
        _compiled[mode] = _build(mode)
    return _compiled[mode]


def _prep_inputs(x, kernels, mode):
    x = np.asarray(x, dtype=np.float32)
    kernels = np.asarray(kernels, dtype=np.float32)
    x_pad = np.zeros((C_IN, H_PAD, W), np.float32)
    x_pad[:, :H, :] = x
    # lhsT layout: [cin, (group kh kw), cout_in_group]
    w = kernels.reshape(N_GROUPS, 128, C_IN, KH, KW).transpose(2, 0, 3, 4, 1)
    w = np.ascontiguousarray(w).reshape(C_IN, N_GROUPS * N_TAPS, 128)

    if mode == "f32r3":
        x_hi = _round_f32r(x_pad)
        x_lo = x_pad - x_hi
        w_hi = _round_f32r(w)
        w_lo = w - w_hi
        xs = np.stack([x_hi, x_lo])  # (2, C_IN, H_PAD, W)
        ws = np.stack([w_hi, w_lo])  # (2, C_IN, 18, 128)
    else:
        xs = x_pad[None]
        ws = w[None]
        if mode == "bf16":
            import ml_dtypes

            xs = xs.astype(ml_dtypes.bfloat16)
            ws = ws.astype(ml_dtypes.bfloat16)

    in_maps = [
        {
            "x": np.ascontiguousarray(
                xs[:, :, ROWS_PER_CORE * i : ROWS_PER_CORE * i + IN_ROWS, :]
            ),
            "w": ws,
        }
        for i in range(N_CORES)
    ]
    return in_maps


def _gather(results):
    out = np.empty((C_OUT, N_CORES * ROWS_PER_CORE, W_OUT), np.float32)
    for i in range(N_CORES):
        o = results[i]["out"]  # (2, 128, 28, 222)
        r0 = ROWS_PER_CORE * i
        out[:128, r0 : r0 + ROWS_PER_CORE, :] = o[0]
        out[128:, r0 : r0 + ROWS_PER_CORE, :] = o[1]
    return np.ascontiguousarray(out[:, :H_OUT, :])


def _run(x, kernels, mode=None, **spmd_kwargs):
    from concourse.bass_utils import run_bass_kernel_spmd

    mode = mode or MM_MODE
    nc = _get_nc(mode)
    in_maps = _prep_inputs(x, kernels, mode)
    res = run_bass_kernel_spmd(nc, in_maps, list(range(N_CORES)), **spmd_kwargs)
    return _gather(res.results), res


def kernel(x, kernels):
    out, _ = _run(x, kernels)
    return out



# revision 2
# speedup vs baseline: 3.8289x; 3.8289x over previous
"""Trainium2 Bass kernel: 3x3 valid cross-correlation (dense CNN layer).

  x:       (128, 224, 224) f32   (C_in, H, W)
  kernels: (256, 128, 3, 3) f32  (C_out, C_in, KH, KW)
  out:     (256, 222, 222) f32   (C_out, H_out, W_out)

Sharding: output rows spatially across the 8 NeuronCores (28 rows per core;
8*28 = 224 >= 222, tail rows computed from zero padding and dropped on
gather). Every core holds the full filter bank. C_in = 128 is exactly the PE
contraction dim; output channels form two 128-partition groups. For each
(row-pair, channel-group) a PSUM tile (128, 2, 222) accumulates one matmul
per filter tap, the moving operand being a shifted window of the SBUF-
resident input slab.

Precision modes (CONV_MM_MODE):
  f32r3 (default): fp32r hi/lo split. fp32r is fp32 RNE-rounded to 11
      explicit mantissa bits but streams at full PE rate; the matmul is
      exact (fp32 PSUM accumulate) on fp32r-representable values. With
      x = x_hi + x_lo and w = w_hi + w_lo (both halves exactly
      representable), 3 passes (hi*hi + hi*lo + lo*hi) give full-fp32
      accuracy at 3 cycles/row vs plain fp32's 4.
  f32r: single-pass fp32r (~1.5e-4 rel err, 1 cycle/row).
  f32:  plain fp32 matmul (4 cycles/row).
  bf16: single-pass bf16 (~1e-2 rel err, 1 cycle/row).
"""

import os
from contextlib import ExitStack

import numpy as np

C_IN, H, W = 128, 224, 224
C_OUT, KH, KW = 256, 3, 3
H_OUT = H - KH + 1  # 222
W_OUT = W - KW + 1  # 222
N_CORES = 8
ROWS_PER_CORE = 28
IN_ROWS = ROWS_PER_CORE + KH - 1  # 30
PAIRS = ROWS_PER_CORE // 2  # 14
N_GROUPS = C_OUT // 128  # 2
H_PAD = N_CORES * ROWS_PER_CORE + KH - 1  # 226
N_TAPS = KH * KW  # 9

MM_MODE = os.environ.get("CONV_MM_MODE", "f32r")

_compiled = {}


def _round_f32r(a):
    """fp32 -> nearest fp32r (low 12 mantissa bits dropped, RNE) — the exact
    rounding trn2 applies when data is ingested as dt.float32r."""
    b = a.view(np.uint32).astype(np.uint64)
    q = np.uint64(1 << 12)
    r = (b + ((q >> np.uint64(1)) - np.uint64(1) + ((b >> np.uint64(12)) & np.uint64(1)))) & ~(q - np.uint64(1))
    return r.astype(np.uint32).view(np.float32)


DEFAULT_CFG = dict(
    xp_bufs=1,
    wp_bufs=1,
    op_bufs=8,
    pp_bufs=8,
    # term-major matmul order + interleaved hi/lo input chunks + per-group w
    # chunks minimize the pipeline-fill stall at kernel start (the first 9
    # matmuls only need w_hi[g0] and the first x chunks). Sustained slope is
    # PE-bound and config-insensitive; these help the single-shot case.
    term_major=True,
    x_chunk=6,
    w_group_chunks=True,
    x_h_outer=False,
    in_dma_gpsimd=False,  # issue input DMAs from gpsimd (separate queues from output)
    pair_block=0,  # >0: tap-major over a block of row-pairs sharing each weight
    # ablation flags (repeat-loop timing experiments)
    load_in_loop=True,  # False: hoist x/w DMA out of the repeat loop
    do_copy=True,  # False: skip psum->sbuf copy except an anchor on the last tile
    do_store=True,  # False: skip output DMA
)


def _build(mm_mode, repeat=1, **cfg_over):
    import concourse.mybir as mybir
    import concourse.tile as tile
    from concourse import bacc

    cfg = {**DEFAULT_CFG, **cfg_over}
    dt = mybir.dt
    split = mm_mode == "f32r3"
    mm_dt = {
        "f32r3": dt.float32r,
        "f32r": dt.float32r,
        "f32": dt.float32,
        "bf16": dt.bfloat16,
    }[mm_mode]
    n_half = 2 if split else 1  # hi/lo copies of x and w

    nc = bacc.Bacc("TRN2", target_bir_lowering=False)
    x_d = nc.dram_tensor(
        "x", [n_half, C_IN, IN_ROWS, W], mm_dt, kind="ExternalInput"
    ).ap()
    w_d = nc.dram_tensor(
        "w", [n_half, C_IN, N_GROUPS * N_TAPS, 128], mm_dt, kind="ExternalInput"
    ).ap()
    o_d = nc.dram_tensor(
        "out", [N_GROUPS, 128, ROWS_PER_CORE, W_OUT], dt.float32, kind="ExternalOutput"
    ).ap()

    def load(nc, tc, xp, wp):
        in_eng = nc.gpsimd if cfg["in_dma_gpsimd"] else nc.sync
        w_sb = wp.tile([C_IN, n_half * N_GROUPS * N_TAPS, 128], mm_dt, name="w_sb")
        if cfg["w_group_chunks"]:
            for h in range(n_half):
                for g in range(N_GROUPS):
                    in_eng.dma_start(
                        w_sb[
                            :,
                            h * N_GROUPS * N_TAPS + g * N_TAPS : h * N_GROUPS * N_TAPS
                            + (g + 1) * N_TAPS,
                            :,
                        ],
                        w_d[h, :, g * N_TAPS : (g + 1) * N_TAPS, :],
                    )
        else:
            for h in range(n_half):
                in_eng.dma_start(
                    w_sb[:, h * N_GROUPS * N_TAPS : (h + 1) * N_GROUPS * N_TAPS, :],
                    w_d[h],
                )
        x_sb = xp.tile([C_IN, n_half * IN_ROWS, W], mm_dt, name="x_sb")
        x_chunk = cfg["x_chunk"]
        if cfg["x_h_outer"]:
            for h in range(n_half):
                for r0 in range(0, IN_ROWS, x_chunk):
                    r1 = min(r0 + x_chunk, IN_ROWS)
                    in_eng.dma_start(
                        x_sb[:, h * IN_ROWS + r0 : h * IN_ROWS + r1, :],
                        x_d[h, :, r0:r1, :],
                    )
        else:
            for r0 in range(0, IN_ROWS, x_chunk):
                r1 = min(r0 + x_chunk, IN_ROWS)
                for h in range(n_half):
                    in_eng.dma_start(
                        x_sb[:, h * IN_ROWS + r0 : h * IN_ROWS + r1, :],
                        x_d[h, :, r0:r1, :],
                    )
        return w_sb, x_sb

    def compute(nc, tc, op, pp, w_sb, x_sb):
        # matmul passes per tap: (w_half, x_half)
        terms = [(0, 0), (0, 1), (1, 0)] if split else [(0, 0)]
        n_mm = len(terms) * N_TAPS
        taps = [(kh, kw) for kh in range(KH) for kw in range(KW)]
        if cfg["term_major"]:
            mm_order = [(wh, xh, kh, kw) for (wh, xh) in terms for (kh, kw) in taps]
        else:
            mm_order = [(wh, xh, kh, kw) for (kh, kw) in taps for (wh, xh) in terms]

        def emit_mm(ps, p, g, wh, xh, kh, kw, start, stop):
            nc.tensor.matmul(
                ps[:],
                w_sb[:, wh * N_GROUPS * N_TAPS + (g * KH + kh) * KW + kw, :],
                x_sb[
                    :,
                    xh * IN_ROWS + 2 * p + kh : xh * IN_ROWS + 2 * p + kh + 2,
                    kw : kw + W_OUT,
                ],
                start=start,
                stop=stop,
            )

        def emit_out(ps, p, g, last):
            if cfg["do_copy"] or last:
                ot = op.tile([128, 2, W_OUT], dt.float32, name="ot")
                nc.vector.tensor_copy(ot[:], ps[:])
                if cfg["do_store"] or last:
                    nc.sync.dma_start(o_d[g, :, 2 * p : 2 * p + 2, :], ot[:])

        B = cfg["pair_block"]
        if B:
            # consecutive matmuls share one stationary weight across B pairs
            for g in range(N_GROUPS):
                for b0 in range(0, PAIRS, B):
                    blk = list(range(b0, min(b0 + B, PAIRS)))
                    tiles = {
                        p: pp.tile([128, 2, W_OUT], dt.float32, name="ps") for p in blk
                    }
                    for i_mm, (wh, xh, kh, kw) in enumerate(mm_order):
                        for p in blk:
                            emit_mm(
                                tiles[p], p, g, wh, xh, kh, kw,
                                i_mm == 0, i_mm == n_mm - 1,
                            )
                    for p in blk:
                        emit_out(
                            tiles[p], p, g,
                            p == PAIRS - 1 and g == N_GROUPS - 1,
                        )
        else:
            for p in range(PAIRS):
                for g in range(N_GROUPS):
                    ps = pp.tile([128, 2, W_OUT], dt.float32, name="ps")
                    for i_mm, (wh, xh, kh, kw) in enumerate(mm_order):
                        emit_mm(ps, p, g, wh, xh, kh, kw, i_mm == 0, i_mm == n_mm - 1)
                    emit_out(ps, p, g, p == PAIRS - 1 and g == N_GROUPS - 1)

    with tile.TileContext(nc) as tc, ExitStack() as ctx:
        xp = ctx.enter_context(tc.tile_pool(name="xp", bufs=cfg["xp_bufs"]))
        wp = ctx.enter_context(tc.tile_pool(name="wp", bufs=cfg["wp_bufs"]))
        op = ctx.enter_context(tc.tile_pool(name="op", bufs=cfg["op_bufs"]))
        pp = ctx.enter_context(
            tc.tile_pool(name="pp", bufs=cfg["pp_bufs"], space="PSUM")
        )
        if repeat == 1:
            w_sb, x_sb = load(nc, tc, xp, wp)
            compute(nc, tc, op, pp, w_sb, x_sb)
        elif cfg["load_in_loop"]:
            with tc.For_i(0, repeat, 1):
                w_sb, x_sb = load(nc, tc, xp, wp)
                compute(nc, tc, op, pp, w_sb, x_sb)
        else:
            w_sb, x_sb = load(nc, tc, xp, wp)
            with tc.For_i(0, repeat, 1):
                compute(nc, tc, op, pp, w_sb, x_sb)

    nc.compile()
    return nc


def _get_nc(mode):
    if mode not in _compiled:
        _compiled[mode] = _build(mode)
    return _compiled[mode]


def _prep_inputs(x, kernels, mode):
    x = np.asarray(x, dtype=np.float32)
    kernels = np.asarray(kernels, dtype=np.float32)
    x_pad = np.zeros((C_IN, H_PAD, W), np.float32)
    x_pad[:, :H, :] = x
    # lhsT layout: [cin, (group kh kw), cout_in_group]
    w = kernels.reshape(N_GROUPS, 128, C_IN, KH, KW).transpose(2, 0, 3, 4, 1)
    w = np.ascontiguousarray(w).reshape(C_IN, N_GROUPS * N_TAPS, 128)

    if mode == "f32r3":
        x_hi = _round_f32r(x_pad)
        x_lo = x_pad - x_hi
        w_hi = _round_f32r(w)
        w_lo = w - w_hi
        xs = np.stack([x_hi, x_lo])  # (2, C_IN, H_PAD, W)
        ws = np.stack([w_hi, w_lo])  # (2, C_IN, 18, 128)
    else:
        xs = x_pad[None]
        ws = w[None]
        if mode == "bf16":
            import ml_dtypes

            xs = xs.astype(ml_dtypes.bfloat16)
            ws = ws.astype(ml_dtypes.bfloat16)

    in_maps = [
        {
            "x": np.ascontiguousarray(
                xs[:, :, ROWS_PER_CORE * i : ROWS_PER_CORE * i + IN_ROWS, :]
            ),
            "w": ws,
        }
        for i in range(N_CORES)
    ]
    return in_maps


def _gather(results):
    out = np.empty((C_OUT, N_CORES * ROWS_PER_CORE, W_OUT), np.float32)
    for i in range(N_CORES):
        o = results[i]["out"]  # (2, 128, 28, 222)
        r0 = ROWS_PER_CORE * i
        out[:128, r0 : r0 + ROWS_PER_CORE, :] = o[0]
        out[128:, r0 : r0 + ROWS_PER_CORE, :] = o[1]
    return np.ascontiguousarray(out[:, :H_OUT, :])


def _run(x, kernels, mode=None, **spmd_kwargs):
    from concourse.bass_utils import run_bass_kernel_spmd

    mode = mode or MM_MODE
    nc = _get_nc(mode)
    in_maps = _prep_inputs(x, kernels, mode)
    res = run_bass_kernel_spmd(nc, in_maps, list(range(N_CORES)), **spmd_kwargs)
    return _gather(res.results), res


def kernel(x, kernels):
    out, _ = _run(x, kernels)
    return out



# revision 14
# speedup vs baseline: 5.2859x; 1.3805x over previous
"""Trainium2 Bass kernel: 3x3 valid cross-correlation (dense CNN layer).

  x:       (128, 224, 224) f32   (C_in, H, W)
  kernels: (256, 128, 3, 3) f32  (C_out, C_in, KH, KW)
  out:     (256, 222, 222) f32   (C_out, H_out, W_out)

Sharding: output rows spatially across the 8 NeuronCores (28 rows per core;
8*28 = 224 >= 222, tail rows computed from zero padding and dropped on
gather). Every core holds the full filter bank. C_in = 128 is exactly the PE
contraction dim; output channels form two 128-partition groups. For each
(row-pair, channel-group) a PSUM tile (128, 2, 222) accumulates one matmul
per filter tap, the moving operand being a shifted window of the SBUF-
resident input slab.

Precision modes (CONV_MM_MODE):
  f32r3 (default): fp32r hi/lo split. fp32r is fp32 RNE-rounded to 11
      explicit mantissa bits but streams at full PE rate; the matmul is
      exact (fp32 PSUM accumulate) on fp32r-representable values. With
      x = x_hi + x_lo and w = w_hi + w_lo (both halves exactly
      representable), 3 passes (hi*hi + hi*lo + lo*hi) give full-fp32
      accuracy at 3 cycles/row vs plain fp32's 4.
  f32r: single-pass fp32r (~1.5e-4 rel err, 1 cycle/row).
  f32:  plain fp32 matmul (4 cycles/row).
  bf16: single-pass bf16 (~1e-2 rel err, 1 cycle/row).
"""

import os
from contextlib import ExitStack

import numpy as np

C_IN, H, W = 128, 224, 224
C_OUT, KH, KW = 256, 3, 3
H_OUT = H - KH + 1  # 222
W_OUT = W - KW + 1  # 222
N_CORES = 8
ROWS_PER_CORE = 28
IN_ROWS = ROWS_PER_CORE + KH - 1  # 30
PAIRS = ROWS_PER_CORE // 2  # 14
N_GROUPS = C_OUT // 128  # 2
H_PAD = N_CORES * ROWS_PER_CORE + KH - 1  # 226
N_TAPS = KH * KW  # 9

MM_MODE = os.environ.get("CONV_MM_MODE", "f32r")

_compiled = {}


def _round_f32r(a):
    """fp32 -> nearest fp32r (low 12 mantissa bits dropped, RNE) — the exact
    rounding trn2 applies when data is ingested as dt.float32r."""
    b = a.view(np.uint32).astype(np.uint64)
    q = np.uint64(1 << 12)
    r = (b + ((q >> np.uint64(1)) - np.uint64(1) + ((b >> np.uint64(12)) & np.uint64(1)))) & ~(q - np.uint64(1))
    return r.astype(np.uint32).view(np.float32)


DEFAULT_CFG = dict(
    xp_bufs=2,  # double-buffer the x chunk tiles across loop iterations
    wp_bufs=2,
    op_bufs=4,  # block staging tiles (PSUM copies land here, then one big DMA)
    pp_bufs=8,
    term_major=True,
    w_eng="sync",  # w on the HWDGE ring (fast first-byte, needed earliest)
    x_eng="gpsimd",  # x chunks on the SWDGE ring, parallel to w
    prewarm=3,  # dummy fp32 matmuls at kernel start: opens the HAM clock gate
    # (PE 1.2->2.4GHz needs ~3.4us of busy) during the input-DMA fill phase
    # output batching: pairs per flush block (sums to PAIRS). Small last block
    # keeps the end-of-kernel DMA tail short; blocks alternate the out rings.
    store_pairs=(5, 5, 3, 1),
    out_engines=("sync",),
    tap_major=False,  # share each stationary weight across a store block's pairs
    staggered=False,  # For_i staggered_reset (cheaper loop back-edge)
    # ablation flags (repeat-loop timing experiments)
    load_in_loop=True,  # False: hoist x/w DMA out of the repeat loop
    do_copy=True,  # False: skip psum->sbuf copy except an anchor on the last tile
    do_store=True,  # False: skip output DMA except the anchor block
)


def _build(mm_mode, repeat=1, **cfg_over):
    import concourse.mybir as mybir
    import concourse.tile as tile
    from concourse import bacc

    cfg = {**DEFAULT_CFG, **cfg_over}
    dt = mybir.dt
    split = mm_mode == "f32r3"
    if split:
        # hi/lo copies double the x/w tiles; shrink buffering to fit SBUF
        cfg["xp_bufs"] = min(cfg["xp_bufs"], 1)
        cfg["op_bufs"] = min(cfg["op_bufs"], 3)
    x_dt, w_dt = {
        "f32r3": (dt.float32r, dt.float32r),
        "f32r": (dt.float32r, dt.float32r),
        "wbf16": (dt.float32r, dt.bfloat16),  # fast FWL weight loads, f32r x
        "f32": (dt.float32, dt.float32),
        "bf16": (dt.bfloat16, dt.bfloat16),
    }[mm_mode]
    n_half = 2 if split else 1  # hi/lo copies of x and w

    nc = bacc.Bacc("TRN2", target_bir_lowering=False)
    x_d = nc.dram_tensor(
        "x", [n_half, C_IN, IN_ROWS, W], x_dt, kind="ExternalInput"
    ).ap()
    w_d = nc.dram_tensor(
        "w", [n_half, C_IN, N_GROUPS * N_TAPS, 128], w_dt, kind="ExternalInput"
    ).ap()
    o_d = nc.dram_tensor(
        "out", [N_GROUPS, 128, ROWS_PER_CORE, W_OUT], dt.float32, kind="ExternalOutput"
    ).ap()

    # x chunk tiles: 8 rows with a 2-row overlap so every pair's 4-row window
    # lives entirely inside one chunk (pair p -> chunk p//3). Separate tiles
    # mean a matmul only waits on ITS chunk's DMA (dep tracking is effectively
    # tile-granular), so x streams in under the matmul wave instead of
    # serializing ahead of it.
    X_CHUNKS = [(6 * c, min(6 * c + 8, IN_ROWS)) for c in range((PAIRS + 2) // 3)]

    def load(nc, tc, xp, wp):
        eng_map = {"sync": nc.sync, "scalar": nc.scalar, "gpsimd": nc.gpsimd}
        w_eng = eng_map[cfg["w_eng"]]
        x_eng = eng_map[cfg["x_eng"]]
        w_tiles = []
        for g in range(N_GROUPS):
            w_sb = wp.tile([C_IN, n_half * N_TAPS, 128], w_dt, name=f"w{g}")
            for h in range(n_half):
                w_eng.dma_start(
                    w_sb[:, h * N_TAPS : (h + 1) * N_TAPS, :],
                    w_d[h, :, g * N_TAPS : (g + 1) * N_TAPS, :],
                )
            w_tiles.append(w_sb)
        x_tiles = []
        for c, (r0, r1) in enumerate(X_CHUNKS):
            nr = r1 - r0
            x_sb = xp.tile([C_IN, n_half * nr, W], x_dt, name=f"x{c}")
            for h in range(n_half):
                x_eng.dma_start(
                    x_sb[:, h * nr : (h + 1) * nr, :], x_d[h, :, r0:r1, :]
                )
            x_tiles.append(x_sb)
        return w_tiles, x_tiles

    def compute(nc, tc, op, pp, w_tiles, x_tiles):
        # matmul passes per tap: (w_half, x_half)
        terms = [(0, 0), (0, 1), (1, 0)] if split else [(0, 0)]
        n_mm = len(terms) * N_TAPS
        taps = [(kh, kw) for kh in range(KH) for kw in range(KW)]
        if cfg["term_major"]:
            mm_order = [(wh, xh, kh, kw) for (wh, xh) in terms for (kh, kw) in taps]
        else:
            mm_order = [(wh, xh, kh, kw) for (kh, kw) in taps for (wh, xh) in terms]

        def emit_mm(ps, p, g, wh, xh, kh, kw, start, stop):
            c = p // 3
            nr = X_CHUNKS[c][1] - X_CHUNKS[c][0]
            r = 2 * p - X_CHUNKS[c][0] + kh
            nc.tensor.matmul(
                ps[:],
                w_tiles[g][:, wh * N_TAPS + kh * KW + kw, :],
                x_tiles[c][:, xh * nr + r : xh * nr + r + 2, kw : kw + W_OUT],
                start=start,
                stop=stop,
            )

        eng_map = {"sync": nc.sync, "scalar": nc.scalar, "gpsimd": nc.gpsimd}
        out_engs = [eng_map[e] for e in cfg["out_engines"]]
        assert sum(cfg["store_pairs"]) == PAIRS
        eng_i = 0
        if cfg["prewarm"]:
            # fp32 dummies run 4 cycles/row: ~1.7us each cold, long enough
            # that 2-3 of them span the 3.4us HAM busy window
            wt = op.tile([128, 512], dt.float32, name="warm")
            nc.vector.memset(wt[:], 0.0)
            pw = pp.tile([128, 512], dt.float32, name="ps")
            for i in range(cfg["prewarm"]):
                nc.tensor.matmul(
                    pw[:], wt[:, :128], wt[:],
                    start=i == 0, stop=i == cfg["prewarm"] - 1,
                )
        # group-outer; per flush block the PSUM copies land in one staging
        # tile, then a single big DMA (>=1MB for the large blocks) goes out on
        # an alternating HWDGE ring. The final 1-pair block keeps the
        # end-of-kernel DMA tail short.
        for g in range(N_GROUPS):
            p0 = 0
            for bp in cfg["store_pairs"]:
                last_blk = g == N_GROUPS - 1 and p0 + bp == PAIRS
                st = op.tile([128, 2 * bp, W_OUT], dt.float32, name="st")
                pairs = list(range(p0, p0 + bp))
                if cfg["tap_major"]:
                    # stationary weight shared across the block's pairs:
                    # bp concurrent PSUM accumulations
                    tiles = {
                        p: pp.tile([128, 2, W_OUT], dt.float32, name="ps")
                        for p in pairs
                    }
                    for i_mm, (wh, xh, kh, kw) in enumerate(mm_order):
                        for p in pairs:
                            emit_mm(
                                tiles[p], p, g, wh, xh, kh, kw,
                                i_mm == 0, i_mm == n_mm - 1,
                            )
                    if cfg["do_copy"] or last_blk:
                        for j, p in enumerate(pairs):
                            nc.vector.tensor_copy(
                                st[:, 2 * j : 2 * j + 2, :], tiles[p][:]
                            )
                else:
                    for j, p in enumerate(pairs):
                        last_pair = last_blk and j == bp - 1
                        ps = pp.tile([128, 2, W_OUT], dt.float32, name="ps")
                        for i_mm, (wh, xh, kh, kw) in enumerate(mm_order):
                            emit_mm(
                                ps, p, g, wh, xh, kh, kw, i_mm == 0, i_mm == n_mm - 1
                            )
                        if cfg["do_copy"] or last_pair:
                            nc.vector.tensor_copy(st[:, 2 * j : 2 * j + 2, :], ps[:])
                if cfg["do_store"] or last_blk:
                    out_engs[eng_i % len(out_engs)].dma_start(
                        o_d[g, :, 2 * p0 : 2 * (p0 + bp), :], st[:]
                    )
                    eng_i += 1
                p0 += bp

    with tile.TileContext(nc) as tc, ExitStack() as ctx:
        xp = ctx.enter_context(tc.tile_pool(name="xp", bufs=cfg["xp_bufs"]))
        wp = ctx.enter_context(tc.tile_pool(name="wp", bufs=cfg["wp_bufs"]))
        op = ctx.enter_context(tc.tile_pool(name="op", bufs=cfg["op_bufs"]))
        pp = ctx.enter_context(
            tc.tile_pool(name="pp", bufs=cfg["pp_bufs"], space="PSUM")
        )
        if repeat == 1:
            w_sb, x_sb = load(nc, tc, xp, wp)
            compute(nc, tc, op, pp, w_sb, x_sb)
        elif cfg["load_in_loop"]:
            with tc.For_i(0, repeat, 1, staggered_reset=cfg["staggered"]):
                w_sb, x_sb = load(nc, tc, xp, wp)
                compute(nc, tc, op, pp, w_sb, x_sb)
        else:
            w_sb, x_sb = load(nc, tc, xp, wp)
            with tc.For_i(0, repeat, 1, staggered_reset=cfg["staggered"]):
                compute(nc, tc, op, pp, w_sb, x_sb)

    nc.compile()
    return nc


def _get_nc(mode):
    if mode not in _compiled:
        _compiled[mode] = _build(mode)
    return _compiled[mode]


def _prep_inputs(x, kernels, mode):
    x = np.asarray(x, dtype=np.float32)
    kernels = np.asarray(kernels, dtype=np.float32)
    x_pad = np.zeros((C_IN, H_PAD, W), np.float32)
    x_pad[:, :H, :] = x
    # lhsT layout: [cin, (group kh kw), cout_in_group]
    w = kernels.reshape(N_GROUPS, 128, C_IN, KH, KW).transpose(2, 0, 3, 4, 1)
    w = np.ascontiguousarray(w).reshape(C_IN, N_GROUPS * N_TAPS, 128)

    if mode == "f32r3":
        x_hi = _round_f32r(x_pad)
        x_lo = x_pad - x_hi
        w_hi = _round_f32r(w)
        w_lo = w - w_hi
        xs = np.stack([x_hi, x_lo])  # (2, C_IN, H_PAD, W)
        ws = np.stack([w_hi, w_lo])  # (2, C_IN, 18, 128)
    else:
        xs = x_pad[None]
        ws = w[None]
        if mode in ("bf16", "wbf16"):
            import ml_dtypes

            ws = ws.astype(ml_dtypes.bfloat16)
            if mode == "bf16":
                xs = xs.astype(ml_dtypes.bfloat16)

    in_maps = [
        {
            "x": np.ascontiguousarray(
                xs[:, :, ROWS_PER_CORE * i : ROWS_PER_CORE * i + IN_ROWS, :]
            ),
            "w": ws,
        }
        for i in range(N_CORES)
    ]
    return in_maps


def _gather(results):
    out = np.empty((C_OUT, N_CORES * ROWS_PER_CORE, W_OUT), np.float32)
    for i in range(N_CORES):
        o = results[i]["out"]  # (2, 128, 28, 222)
        r0 = ROWS_PER_CORE * i
        out[:128, r0 : r0 + ROWS_PER_CORE, :] = o[0]
        out[128:, r0 : r0 + ROWS_PER_CORE, :] = o[1]
    return np.ascontiguousarray(out[:, :H_OUT, :])


def _run(x, kernels, mode=None, **spmd_kwargs):
    from concourse.bass_utils import run_bass_kernel_spmd

    mode = mode or MM_MODE
    nc = _get_nc(mode)
    in_maps = _prep_inputs(x, kernels, mode)
    res = run_bass_kernel_spmd(nc, in_maps, list(range(N_CORES)), **spmd_kwargs)
    return _gather(res.results), res


def kernel(x, kernels):
    out, _ = _run(x, kernels)
    return out

